# revision 11
# baseline (speedup 1.0000x reference)
"""Trainium2 Bass kernel for nn_Decoder_31370441129997.

GRU decoder: 12 sequential steps of (Linear+ReLU) -> 3x GRU cell -> Linear(2),
with the input-layer representation fed back from the last GRU layer's hidden.

Strategy: data-parallel over batch (4096 -> 8 cores x 512). All weights
resident in SBUF as bf16 (full PE rate + fast weight loads). Activations kept
feature-major [H, B] so the recurrence needs no transposes. Hidden state is
ping-pong buffered per step parity, so gate outputs write their h tile
directly and no copies are needed. The h-side matmuls (which depend only on
h(t-1)) are software-pipelined two chunks ahead of the x-side matmuls to keep
the PE busy across the gate-latency stalls at layer and step boundaries.
Gate math: ACT sigmoid/tanh with per-partition bias APs (bf16 outputs), DVE
scalar_tensor_tensor for the n-gate preact, GpSimd for the z*h term.
"""
import os
import sys

sys.path.insert(0, "/opt/trn_rl_repo")

from contextlib import ExitStack

import numpy as np

import concourse.bass as bass
import concourse.tile as tile
from concourse import bacc, mybir
from concourse.bass_utils import run_bass_kernel_spmd

TPRED = 12
H = 512
L = 3
B = 4096
NCORES = 8
BL = B // NCORES  # 512 batch rows per core
KT = H // 128     # contraction chunks
MT = H // 128     # feature tiles per gate

F32 = mybir.dt.float32
F32R = mybir.dt.float32r
BF16 = mybir.dt.bfloat16
AF = mybir.ActivationFunctionType
ALU = mybir.AluOpType

# compute dtype for matmul operands: bf16 (~6e-3 error, tolerance is 2e-2)
# or f32r (TF32-like, ~4e-4 error, slower weight loads)
MODE = os.environ.get("KERNEL_DTYPE", "bf16")
CDT = F32R if MODE == "f32r" else BF16
GDT = F32 if MODE == "f32r" else BF16  # gate-value dtype

_CACHE = {}


def _round_f32r(x: np.ndarray) -> np.ndarray:
    """Round fp32 to the PE's float32r grid (RNE, drop 12 low mantissa bits)."""
    u = np.ascontiguousarray(x, dtype=np.float32).view(np.uint32).astype(np.uint64)
    lsb = (u >> np.uint64(12)) & np.uint64(1)
    u = (u + np.uint64(0x7FF) + lsb) & np.uint64(0xFFFFF000)
    return u.astype(np.uint32).view(np.float32).reshape(x.shape)


def _f(t):
    """bitcast f32r tiles to f32 for vector-engine reads."""
    return t[:].bitcast(F32) if CDT is F32R else t[:]


def _build():
    """Build + compile the per-core Bass program (identical on all 8 cores)."""
    nc = bacc.Bacc("TRN2", target_bir_lowering=False, debug=False,
                   dynamic_dma_scratch_size=512)

    rep_d = nc.dram_tensor("rep", [H, BL], CDT, kind="ExternalInput").ap()
    win_d = nc.dram_tensor("win", [128, KT * H], CDT, kind="ExternalInput").ap()
    wx_d = nc.dram_tensor("wx", [L, H, 3 * H], CDT, kind="ExternalInput").ap()
    wh_d = nc.dram_tensor("wh", [L, H, 3 * H], CDT, kind="ExternalInput").ap()
    wout_d = nc.dram_tensor("wout", [H, 2], CDT, kind="ExternalInput").ap()
    bias_d = nc.dram_tensor("bias", [128, 53], F32, kind="ExternalInput").ap()
    out_d = nc.dram_tensor("out", [TPRED, 2, BL], F32, kind="ExternalOutput").ap()

    with tile.TileContext(nc) as tc, ExitStack() as ctx:
        wpool = ctx.enter_context(tc.tile_pool(name="w", bufs=1))
        state = ctx.enter_context(tc.tile_pool(name="state", bufs=1))
        gates = ctx.enter_context(tc.tile_pool(name="gates", bufs=2))
        psum = ctx.enter_context(tc.tile_pool(name="psum", bufs=2, space="PSUM"))

        # ping-pong hidden state: step t writes hb[t%2], reads hb[(t+1)%2].
        # hb[1][2] doubles as the step-0 representation input.
        hb = [[[state.tile([128, BL], CDT, tag=f"h{p}_{l}_{m}",
                           name=f"h{p}_{l}_{m}")
                for m in range(MT)] for l in range(L)] for p in range(2)]

        # Each engine owns one DMA queue moving ~130 GB/s, and descriptor
        # issue costs ~645ns on the issuing engine, so the startup stream is
        # spread need-ordered across all three DMA-capable engines (sync,
        # scalar, gpsimd) to get step 0's rep/W_in/W_ih[0] in flight at once.
        wx = [wpool.tile([128, KT, 3 * H], CDT, tag=f"wx{l}", name=f"wx{l}")
              for l in range(L)]
        wh = [wpool.tile([128, KT, 3 * H], CDT, tag=f"wh{l}", name=f"wh{l}")
              for l in range(L)]
        wout = wpool.tile([128, KT, 2], CDT, tag="wout")
        scratch = state.tile([128, BL], CDT, tag="scratch")

        nc.gpsimd.memset(scratch[:], 0.0)
        for k in (2, 3):
            nc.gpsimd.dma_start(wx[0][:, k, :], wx_d[0, k * 128:(k + 1) * 128, :])
        nc.gpsimd.dma_start(wx[1][:, 3, :], wx_d[1, 3 * 128:4 * 128, :])

        for k in (0, 1):
            nc.scalar.dma_start(wx[0][:, k, :], wx_d[0, k * 128:(k + 1) * 128, :])
        for k in range(KT):
            nc.scalar.dma_start(wx[2][:, k, :], wx_d[2, k * 128:(k + 1) * 128, :])

        # win_d is pre-swizzled host-side to the SBUF image [128, KT*H] so
        # this is one descriptor with 4KB-contiguous runs per partition
        win = wpool.tile([128, KT, H], CDT, tag="win")
        nc.sync.dma_start(win[:], win_d.rearrange("p (kt c) -> p kt c", kt=KT))
        for m in range(2):
            nc.sync.dma_start(hb[1][2][m][:], rep_d[m * 128:(m + 1) * 128, :])
        bias = wpool.tile([128, 53], F32, tag="bias")
        nc.sync.dma_start(bias[:], bias_d[:])
        for m in range(2, MT):
            nc.sync.dma_start(hb[1][2][m][:], rep_d[m * 128:(m + 1) * 128, :])
        for k in range(KT - 1):
            nc.sync.dma_start(wx[1][:, k, :], wx_d[1, k * 128:(k + 1) * 128, :])
        nc.sync.dma_start(wout[:], wout_d.rearrange("(kt p) c -> p kt c", p=128))
        for l in range(L):
            for k in range(KT):
                nc.sync.dma_start(wh[l][:, k, :], wh_d[l, k * 128:(k + 1) * 128, :])

        # warm up the PE clock (DVFS ramps over ~3us of continuous work)
        # while the first DMAs are still in flight
        for _ in range(28):
            pw = psum.tile([128, BL], F32, tag="in")
            nc.tensor.matmul(pw[:], lhsT=scratch[:, 0:128], rhs=scratch[:],
                             start=True, stop=True)

        x = [state.tile([128, BL], CDT, tag=f"x{m}", name=f"x{m}")
             for m in range(MT)]

        def bcol(c):
            return bias[:, c:c + 1]

        def outproj(t):
            # b_out is added host-side after the gather
            po = psum.tile([2, BL], F32, tag="z")
            h2 = hb[t % 2][2]
            for k in range(KT):
                nc.tensor.matmul(po[:], lhsT=wout[:, k, :], rhs=h2[k][:],
                                 start=(k == 0), stop=(k == KT - 1))
            o = gates.tile([2, BL], F32, tag="o")
            nc.scalar.copy(o[:], po[:])
            nc.sync.dma_start(out_d[t], o[:])

        def hside(t, l, m):
            """h-side matmul group for (layer l, chunk m): depends only on
            h(t-1), so it is the fill-in work for gate-latency stalls."""
            hp, whl = hb[(t + 1) % 2][l], wh[l]
            lo, hi = m * 128, (m + 1) * 128
            ph = psum.tile([128, BL], F32, tag="hn", name=f"ph_{t}_{l}_{m}")
            for k in range(KT):
                nc.tensor.matmul(ph[:], lhsT=whl[:, k, 2 * H + lo:2 * H + hi],
                                 rhs=hp[k][:], start=(k == 0), stop=(k == KT - 1))
            pr = psum.tile([128, BL], F32, tag="r", name=f"pr_{t}_{l}_{m}")
            for k in range(KT):
                nc.tensor.matmul(pr[:], lhsT=whl[:, k, lo:hi],
                                 rhs=hp[k][:], start=(k == 0), stop=False)
            pz = psum.tile([128, BL], F32, tag="z", name=f"pz_{t}_{l}_{m}")
            for k in range(KT):
                nc.tensor.matmul(pz[:], lhsT=whl[:, k, H + lo:H + hi],
                                 rhs=hp[k][:], start=(k == 0), stop=False)
            return ph, pr, pz

        for t in range(TPRED):
            pend = {}
            if t > 0:
                # two-chunk hoist across the step boundary: pure h(t-1) work
                # that covers the wait for step t-1's last gate chain
                pend[(0, 0)] = hside(t, 0, 0)
                pend[(0, 1)] = hside(t, 0, 1)
                outproj(t - 1)
            # input layer: x = relu(W_in @ h2 + b_in)
            h2in = hb[(t + 1) % 2][2]
            for m in range(MT):
                px = psum.tile([128, BL], F32, tag="in")
                for k in range(KT):
                    nc.tensor.matmul(px[:],
                                     lhsT=win[:, k, m * 128:(m + 1) * 128],
                                     rhs=h2in[k][:],
                                     start=(k == 0), stop=(k == KT - 1))
                nc.scalar.activation(x[m][:], px[:], AF.Relu, bias=bcol(48 + m))
            for l in range(L):
                xin = x if l == 0 else hb[t % 2][l - 1]
                hold = hb[(t + 1) % 2][l]
                wxl = wx[l]
                for m in range(MT):
                    lo = m * 128
                    hi = lo + 128
                    if t > 0:
                        ph, pr, pz = pend.pop((l, m))
                    else:
                        ph = None
                        pr = psum.tile([128, BL], F32, tag="r",
                                       name=f"pr_{t}_{l}_{m}")
                        pz = psum.tile([128, BL], F32, tag="z",
                                       name=f"pz_{t}_{l}_{m}")
                    for k in range(KT):
                        nc.tensor.matmul(pr[:], lhsT=wxl[:, k, lo:hi],
                                         rhs=xin[k][:],
                                         start=(t == 0 and k == 0),
                                         stop=(k == KT - 1))
                    for k in range(KT):
                        nc.tensor.matmul(pz[:], lhsT=wxl[:, k, H + lo:H + hi],
                                         rhs=xin[k][:],
                                         start=(t == 0 and k == 0),
                                         stop=(k == KT - 1))
                    pin = psum.tile([128, BL], F32, tag="in")
                    for k in range(KT):
                        nc.tensor.matmul(pin[:], lhsT=wxl[:, k, 2 * H + lo:2 * H + hi],
                                         rhs=xin[k][:], start=(k == 0),
                                         stop=(k == KT - 1))
                    # keep the PE two h-side chunks ahead of the x-side: the
                    # next layer's first x-side matmul stalls on this layer's
                    # last gate chain, so 24 insts of h(t-1) work go right
                    # before it (the step boundary gets the same treatment
                    # from the next iteration's top-of-loop hoist).
                    if t > 0:
                        if m < 2:
                            pend[(l, m + 2)] = hside(t, l, m + 2)
                        elif m == 3 and l + 1 < L:
                            pend[(l + 1, 0)] = hside(t, l + 1, 0)
                            pend[(l + 1, 1)] = hside(t, l + 1, 1)

                    # gates for this feature chunk.
                    # h' = e1 + q*n with e1 = z*h, q = 1-z hoisted off the
                    # post-tanh critical chain (t=0: h' = n - z*n).
                    r = gates.tile([128, BL], GDT, tag="r")
                    nc.scalar.activation(r[:], pr[:], AF.Sigmoid,
                                         bias=bcol(l * 16 + m))
                    z = gates.tile([128, BL], GDT, tag="z")
                    nc.scalar.activation(z[:], pz[:], AF.Sigmoid,
                                         bias=bcol(l * 16 + 4 + m))
                    if t > 0:
                        q = gates.tile([128, BL], GDT, tag="q")
                        nc.scalar.activation(q[:], z[:], AF.Identity, bias=1.0,
                                             scale=-1.0)
                        e1 = gates.tile([128, BL], GDT, tag="e1")
                        nc.gpsimd.tensor_mul(e1[:], z[:], _f(hold[m]))
                    t1 = gates.tile([128, BL], F32, tag="t1")
                    if t > 0:
                        # t1 = (hn_psum + b_hh_n) * r
                        nc.vector.scalar_tensor_tensor(
                            t1[:], ph[:], bcol(l * 16 + 8 + m), r[:],
                            op0=ALU.add, op1=ALU.mult)
                    else:
                        nc.vector.tensor_scalar(t1[:], r[:], bcol(l * 16 + 8 + m),
                                                None, op0=ALU.mult)
                    t2 = gates.tile([128, BL], F32, tag="t2")
                    nc.vector.tensor_add(t2[:], t1[:], pin[:])
                    n = gates.tile([128, BL], GDT, tag="n")
                    nc.scalar.activation(n[:], t2[:], AF.Tanh,
                                         bias=bcol(l * 16 + 12 + m))
                    e2 = gates.tile([128, BL], GDT, tag="e2")
                    hdst = hb[t % 2][l][m]
                    if t > 0:
                        nc.vector.tensor_mul(e2[:], q[:], n[:])
                        nc.vector.tensor_add(hdst[:], e1[:], e2[:])
                    else:
                        nc.vector.tensor_mul(e2[:], z[:], n[:])
                        nc.vector.tensor_sub(hdst[:], n[:], e2[:])
        outproj(TPRED - 1)

    nc.compile()
    return nc


def _to_dev(x):
    if CDT is F32R:
        return _round_f32r(x)
    import ml_dtypes
    return np.ascontiguousarray(x).astype(ml_dtypes.bfloat16)


def _prep_inputs(representation, W_in, b_in, W_ih, W_hh, b_ih, b_hh, W_out, b_out):
    rep_T = np.ascontiguousarray(representation.reshape(B, H).T)  # [H, B]
    win = _to_dev(np.ascontiguousarray(
        W_in.T.reshape(KT, 128, H).transpose(1, 0, 2).reshape(128, KT * H)))
    wx = _to_dev(np.ascontiguousarray(np.transpose(W_ih, (0, 2, 1))))
    wh = _to_dev(np.ascontiguousarray(np.transpose(W_hh, (0, 2, 1))))
    wout = _to_dev(np.ascontiguousarray(W_out.T))                 # [H, 2]

    bias = np.zeros((128, 53), dtype=np.float32)
    brz = (b_ih[:, :2 * H] + b_hh[:, :2 * H]).astype(np.float32)  # [L, 2H]
    for l in range(L):
        for g in range(2):
            for m in range(MT):
                bias[:, l * 16 + g * 4 + m] = brz[l, g * H + m * 128:
                                                  g * H + (m + 1) * 128]
        for m in range(MT):
            bias[:, l * 16 + 8 + m] = b_hh[l, 2 * H + m * 128:2 * H + (m + 1) * 128]
            bias[:, l * 16 + 12 + m] = b_ih[l, 2 * H + m * 128:2 * H + (m + 1) * 128]
    for m in range(MT):
        bias[:, 48 + m] = b_in[m * 128:(m + 1) * 128]
    bias[0:2, 52] = b_out

    shared = {"win": win, "wx": wx, "wh": wh, "wout": wout, "bias": bias}
    in_maps = []
    for c in range(NCORES):
        m = dict(shared)
        m["rep"] = _to_dev(np.ascontiguousarray(rep_T[:, c * BL:(c + 1) * BL]))
        in_maps.append(m)
    return in_maps


def _run(inputs, trace=False):
    if "nc" not in _CACHE:
        _CACHE["nc"] = _build()
    nc = _CACHE["nc"]
    in_maps = _prep_inputs(
        inputs["representation"], inputs["W_in"], inputs["b_in"],
        inputs["W_ih"], inputs["W_hh"], inputs["b_ih"], inputs["b_hh"],
        inputs["W_out"], inputs["b_out"])
    res = run_bass_kernel_spmd(nc, in_maps, core_ids=list(range(NCORES)),
                               trace=trace)
    # per-core out: [TPRED, 2, BL] -> full [B, TPRED, 2]
    full = np.empty((B, TPRED, 2), dtype=np.float32)
    for c in range(NCORES):
        o = res.results[c]["out"]                      # [12, 2, BL]
        full[c * BL:(c + 1) * BL] = np.transpose(o, (2, 0, 1))
    full += inputs["b_out"].astype(np.float32)[None, None, :]
    return full, res


def kernel(**inputs) -> np.ndarray:
    out, _ = _run(inputs, trace=False)
    return out


def _setup_tracing():
    """Register the NTFF profile hook shim (test harness only)."""
    import types

    import trn_agent_boot.trn_boot as tb

    mod = types.ModuleType("antenv.axon_hooks")
    hook = [tb._ntff_profile_via_ctypes("/opt/axon/libaxon_pjrt.so")]
    mod.get_axon_ntff_profile_hook = lambda: hook[0]
    mod.set_axon_ntff_profile_hook = lambda h: hook.__setitem__(0, h)
    sys.modules["antenv.axon_hooks"] = mod
    import antenv
    antenv.axon_hooks = mod

    from concourse import bass_utils
    bass_utils.upload_artifacts = lambda tmpdir: str(tmpdir)


# revision 13
# speedup vs baseline: 1.0038x; 1.0038x over previous
"""Trainium2 Bass kernel for nn_Decoder_31370441129997.

GRU decoder: 12 sequential steps of (Linear+ReLU) -> 3x GRU cell -> Linear(2),
with the input-layer representation fed back from the last GRU layer's hidden.

Strategy: data-parallel over batch (4096 -> 8 cores x 512). All weights
resident in SBUF as bf16 (full PE rate + fast weight loads). Activations kept
feature-major [H, B] so the recurrence needs no transposes. Hidden state is
ping-pong buffered per step parity, so gate outputs write their h tile
directly and no copies are needed. The h-side matmuls (which depend only on
h(t-1)) are software-pipelined two chunks ahead of the x-side matmuls to keep
the PE busy across the gate-latency stalls at layer and step boundaries.
Gate math: ACT sigmoid/tanh with per-partition bias APs (bf16 outputs), DVE
scalar_tensor_tensor for the n-gate preact, GpSimd for the z*h term.
"""
import os
import sys

sys.path.insert(0, "/opt/trn_rl_repo")

from contextlib import ExitStack

import numpy as np

import concourse.bass as bass
import concourse.tile as tile
from concourse import bacc, mybir
from concourse.bass_utils import run_bass_kernel_spmd

TPRED = 12
H = 512
L = 3
B = 4096
NCORES = 8
BL = B // NCORES  # 512 batch rows per core
KT = H // 128     # contraction chunks
MT = H // 128     # feature tiles per gate

F32 = mybir.dt.float32
F32R = mybir.dt.float32r
BF16 = mybir.dt.bfloat16
AF = mybir.ActivationFunctionType
ALU = mybir.AluOpType

# compute dtype for matmul operands: bf16 (~6e-3 error, tolerance is 2e-2)
# or f32r (TF32-like, ~4e-4 error, slower weight loads)
MODE = os.environ.get("KERNEL_DTYPE", "bf16")
CDT = F32R if MODE == "f32r" else BF16
GDT = F32 if MODE == "f32r" else BF16  # gate-value dtype

_CACHE = {}


def _round_f32r(x: np.ndarray) -> np.ndarray:
    """Round fp32 to the PE's float32r grid (RNE, drop 12 low mantissa bits)."""
    u = np.ascontiguousarray(x, dtype=np.float32).view(np.uint32).astype(np.uint64)
    lsb = (u >> np.uint64(12)) & np.uint64(1)
    u = (u + np.uint64(0x7FF) + lsb) & np.uint64(0xFFFFF000)
    return u.astype(np.uint32).view(np.float32).reshape(x.shape)


def _f(t):
    """bitcast f32r tiles to f32 for vector-engine reads."""
    return t[:].bitcast(F32) if CDT is F32R else t[:]


def _build():
    """Build + compile the per-core Bass program (identical on all 8 cores)."""
    nc = bacc.Bacc("TRN2", target_bir_lowering=False, debug=False,
                   dynamic_dma_scratch_size=512)

    rep_d = nc.dram_tensor("rep", [H, BL], CDT, kind="ExternalInput").ap()
    win_d = nc.dram_tensor("win", [128, KT * H], CDT, kind="ExternalInput").ap()
    wx_d = nc.dram_tensor("wx", [L, H, 3 * H], CDT, kind="ExternalInput").ap()
    wh_d = nc.dram_tensor("wh", [L, H, 3 * H], CDT, kind="ExternalInput").ap()
    wout_d = nc.dram_tensor("wout", [H, 2], CDT, kind="ExternalInput").ap()
    bias_d = nc.dram_tensor("bias", [128, 53], F32, kind="ExternalInput").ap()
    out_d = nc.dram_tensor("out", [TPRED, 2, BL], F32, kind="ExternalOutput").ap()

    with tile.TileContext(nc) as tc, ExitStack() as ctx:
        wpool = ctx.enter_context(tc.tile_pool(name="w", bufs=1))
        state = ctx.enter_context(tc.tile_pool(name="state", bufs=1))
        gates = ctx.enter_context(tc.tile_pool(name="gates", bufs=2))
        psum = ctx.enter_context(tc.tile_pool(name="psum", bufs=2, space="PSUM"))

        # ping-pong hidden state: step t writes hb[t%2], reads hb[(t+1)%2].
        # hb[1][2] doubles as the step-0 representation input.
        hb = [[[state.tile([128, BL], CDT, tag=f"h{p}_{l}_{m}",
                           name=f"h{p}_{l}_{m}")
                for m in range(MT)] for l in range(L)] for p in range(2)]

        # Each engine owns one DMA queue moving ~130 GB/s, and descriptor
        # issue costs ~645ns on the issuing engine, so the startup stream is
        # spread need-ordered across all three DMA-capable engines (sync,
        # scalar, gpsimd) to get step 0's rep/W_in/W_ih[0] in flight at once.
        wx = [wpool.tile([128, KT, 3 * H], CDT, tag=f"wx{l}", name=f"wx{l}")
              for l in range(L)]
        wh = [wpool.tile([128, KT, 3 * H], CDT, tag=f"wh{l}", name=f"wh{l}")
              for l in range(L)]
        wout = wpool.tile([128, KT, 2], CDT, tag="wout")
        scratch = state.tile([128, BL], CDT, tag="scratch")

        nc.gpsimd.memset(scratch[:], 0.0)
        for k in (2, 3):
            nc.gpsimd.dma_start(wx[0][:, k, :], wx_d[0, k * 128:(k + 1) * 128, :])
        nc.gpsimd.dma_start(wx[1][:, 3, :], wx_d[1, 3 * 128:4 * 128, :])

        for k in (0, 1):
            nc.scalar.dma_start(wx[0][:, k, :], wx_d[0, k * 128:(k + 1) * 128, :])
        for k in range(KT):
            nc.scalar.dma_start(wx[2][:, k, :], wx_d[2, k * 128:(k + 1) * 128, :])

        # bias first (tiny), then (rep chunk k, win chunk k) pairs so the
        # input-layer matmuls unblock progressively as transfers land
        bias = wpool.tile([128, 53], F32, tag="bias")
        nc.sync.dma_start(bias[:], bias_d[:])
        win = wpool.tile([128, KT, H], CDT, tag="win")
        for k in range(KT):
            nc.sync.dma_start(hb[1][2][k][:], rep_d[k * 128:(k + 1) * 128, :])
            nc.sync.dma_start(win[:, k, :],
                              win_d[:, k * H:(k + 1) * H])
        for k in range(KT - 1):
            nc.sync.dma_start(wx[1][:, k, :], wx_d[1, k * 128:(k + 1) * 128, :])
        nc.sync.dma_start(wout[:], wout_d.rearrange("(kt p) c -> p kt c", p=128))
        for l in range(L):
            for k in range(KT):
                nc.sync.dma_start(wh[l][:, k, :], wh_d[l, k * 128:(k + 1) * 128, :])

        # warm up the PE clock (DVFS ramps over ~3us of continuous work)
        # while the first DMAs are still in flight
        for _ in range(16):
            pw = psum.tile([128, BL], F32, tag="in")
            nc.tensor.matmul(pw[:], lhsT=scratch[:, 0:128], rhs=scratch[:],
                             start=True, stop=True)

        x = [state.tile([128, BL], CDT, tag=f"x{m}", name=f"x{m}")
             for m in range(MT)]

        def bcol(c):
            return bias[:, c:c + 1]

        def outproj(t):
            # b_out is added host-side after the gather
            po = psum.tile([2, BL], F32, tag="z")
            h2 = hb[t % 2][2]
            for k in range(KT):
                nc.tensor.matmul(po[:], lhsT=wout[:, k, :], rhs=h2[k][:],
                                 start=(k == 0), stop=(k == KT - 1))
            o = gates.tile([2, BL], F32, tag="o")
            nc.scalar.copy(o[:], po[:])
            nc.sync.dma_start(out_d[t], o[:])

        def hside(t, l, m):
            """h-side matmul group for (layer l, chunk m): depends only on
            h(t-1), so it is the fill-in work for gate-latency stalls."""
            hp, whl = hb[(t + 1) % 2][l], wh[l]
            lo, hi = m * 128, (m + 1) * 128
            ph = psum.tile([128, BL], F32, tag="hn", name=f"ph_{t}_{l}_{m}")
            for k in range(KT):
                nc.tensor.matmul(ph[:], lhsT=whl[:, k, 2 * H + lo:2 * H + hi],
                                 rhs=hp[k][:], start=(k == 0), stop=(k == KT - 1))
            pr = psum.tile([128, BL], F32, tag="r", name=f"pr_{t}_{l}_{m}")
            for k in range(KT):
                nc.tensor.matmul(pr[:], lhsT=whl[:, k, lo:hi],
                                 rhs=hp[k][:], start=(k == 0), stop=False)
            pz = psum.tile([128, BL], F32, tag="z", name=f"pz_{t}_{l}_{m}")
            for k in range(KT):
                nc.tensor.matmul(pz[:], lhsT=whl[:, k, H + lo:H + hi],
                                 rhs=hp[k][:], start=(k == 0), stop=False)
            return ph, pr, pz

        for t in range(TPRED):
            pend = {}
            if t > 0:
                # two-chunk hoist across the step boundary: pure h(t-1) work
                # that covers the wait for step t-1's last gate chain
                pend[(0, 0)] = hside(t, 0, 0)
                pend[(0, 1)] = hside(t, 0, 1)
                outproj(t - 1)
            # input layer: x = relu(W_in @ h2 + b_in)
            h2in = hb[(t + 1) % 2][2]
            for m in range(MT):
                px = psum.tile([128, BL], F32, tag="in")
                for k in range(KT):
                    nc.tensor.matmul(px[:],
                                     lhsT=win[:, k, m * 128:(m + 1) * 128],
                                     rhs=h2in[k][:],
                                     start=(k == 0), stop=(k == KT - 1))
                nc.scalar.activation(x[m][:], px[:], AF.Relu, bias=bcol(48 + m))
            for l in range(L):
                xin = x if l == 0 else hb[t % 2][l - 1]
                hold = hb[(t + 1) % 2][l]
                wxl = wx[l]
                for m in range(MT):
                    lo = m * 128
                    hi = lo + 128
                    if t > 0:
                        ph, pr, pz = pend.pop((l, m))
                    else:
                        ph = None
                        pr = psum.tile([128, BL], F32, tag="r",
                                       name=f"pr_{t}_{l}_{m}")
                        pz = psum.tile([128, BL], F32, tag="z",
                                       name=f"pz_{t}_{l}_{m}")
                    for k in range(KT):
                        nc.tensor.matmul(pr[:], lhsT=wxl[:, k, lo:hi],
                                         rhs=xin[k][:],
                                         start=(t == 0 and k == 0),
                                         stop=(k == KT - 1))
                    for k in range(KT):
                        nc.tensor.matmul(pz[:], lhsT=wxl[:, k, H + lo:H + hi],
                                         rhs=xin[k][:],
                                         start=(t == 0 and k == 0),
                                         stop=(k == KT - 1))
                    pin = psum.tile([128, BL], F32, tag="in")
                    for k in range(KT):
                        nc.tensor.matmul(pin[:], lhsT=wxl[:, k, 2 * H + lo:2 * H + hi],
                                         rhs=xin[k][:], start=(k == 0),
                                         stop=(k == KT - 1))
                    # keep the PE two h-side chunks ahead of the x-side: the
                    # next layer's first x-side matmul stalls on this layer's
                    # last gate chain, so 24 insts of h(t-1) work go right
                    # before it (the step boundary gets the same treatment
                    # from the next iteration's top-of-loop hoist).
                    if t > 0:
                        if m < 2:
                            pend[(l, m + 2)] = hside(t, l, m + 2)
                        elif m == 3 and l + 1 < L:
                            pend[(l + 1, 0)] = hside(t, l + 1, 0)
                            pend[(l + 1, 1)] = hside(t, l + 1, 1)

                    # gates for this feature chunk.
                    # h' = e1 + q*n with e1 = z*h, q = 1-z hoisted off the
                    # post-tanh critical chain (t=0: h' = n - z*n).
                    r = gates.tile([128, BL], GDT, tag="r")
                    nc.scalar.activation(r[:], pr[:], AF.Sigmoid,
                                         bias=bcol(l * 16 + m))
                    z = gates.tile([128, BL], GDT, tag="z")
                    nc.scalar.activation(z[:], pz[:], AF.Sigmoid,
                                         bias=bcol(l * 16 + 4 + m))
                    if t > 0:
                        q = gates.tile([128, BL], GDT, tag="q")
                        nc.scalar.activation(q[:], z[:], AF.Identity, bias=1.0,
                                             scale=-1.0)
                        e1 = gates.tile([128, BL], GDT, tag="e1")
                        nc.gpsimd.tensor_mul(e1[:], z[:], _f(hold[m]))
                    t1 = gates.tile([128, BL], F32, tag="t1")
                    if t > 0:
                        # t1 = (hn_psum + b_hh_n) * r
                        nc.vector.scalar_tensor_tensor(
                            t1[:], ph[:], bcol(l * 16 + 8 + m), r[:],
                            op0=ALU.add, op1=ALU.mult)
                    else:
                        nc.vector.tensor_scalar(t1[:], r[:], bcol(l * 16 + 8 + m),
                                                None, op0=ALU.mult)
                    t2 = gates.tile([128, BL], F32, tag="t2")
                    nc.vector.tensor_add(t2[:], t1[:], pin[:])
                    n = gates.tile([128, BL], GDT, tag="n")
                    nc.scalar.activation(n[:], t2[:], AF.Tanh,
                                         bias=bcol(l * 16 + 12 + m))
                    e2 = gates.tile([128, BL], GDT, tag="e2")
                    hdst = hb[t % 2][l][m]
                    if t > 0:
                        nc.vector.tensor_mul(e2[:], q[:], n[:])
                        nc.vector.tensor_add(hdst[:], e1[:], e2[:])
                    else:
                        nc.vector.tensor_mul(e2[:], z[:], n[:])
                        nc.vector.tensor_sub(hdst[:], n[:], e2[:])
        outproj(TPRED - 1)

    nc.compile()
    return nc


def _to_dev(x):
    if CDT is F32R:
        return _round_f32r(x)
    import ml_dtypes
    return np.ascontiguousarray(x).astype(ml_dtypes.bfloat16)


def _prep_inputs(representation, W_in, b_in, W_ih, W_hh, b_ih, b_hh, W_out, b_out):
    rep_T = np.ascontiguousarray(representation.reshape(B, H).T)  # [H, B]
    win = _to_dev(np.ascontiguousarray(
        W_in.T.reshape(KT, 128, H).transpose(1, 0, 2).reshape(128, KT * H)))
    wx = _to_dev(np.ascontiguousarray(np.transpose(W_ih, (0, 2, 1))))
    wh = _to_dev(np.ascontiguousarray(np.transpose(W_hh, (0, 2, 1))))
    wout = _to_dev(np.ascontiguousarray(W_out.T))                 # [H, 2]

    bias = np.zeros((128, 53), dtype=np.float32)
    brz = (b_ih[:, :2 * H] + b_hh[:, :2 * H]).astype(np.float32)  # [L, 2H]
    for l in range(L):
        for g in range(2):
            for m in range(MT):
                bias[:, l * 16 + g * 4 + m] = brz[l, g * H + m * 128:
                                                  g * H + (m + 1) * 128]
        for m in range(MT):
            bias[:, l * 16 + 8 + m] = b_hh[l, 2 * H + m * 128:2 * H + (m + 1) * 128]
            bias[:, l * 16 + 12 + m] = b_ih[l, 2 * H + m * 128:2 * H + (m + 1) * 128]
    for m in range(MT):
        bias[:, 48 + m] = b_in[m * 128:(m + 1) * 128]
    bias[0:2, 52] = b_out

    shared = {"win": win, "wx": wx, "wh": wh, "wout": wout, "bias": bias}
    in_maps = []
    for c in range(NCORES):
        m = dict(shared)
        m["rep"] = _to_dev(np.ascontiguousarray(rep_T[:, c * BL:(c + 1) * BL]))
        in_maps.append(m)
    return in_maps


def _run(inputs, trace=False):
    if "nc" not in _CACHE:
        _CACHE["nc"] = _build()
    nc = _CACHE["nc"]
    in_maps = _prep_inputs(
        inputs["representation"], inputs["W_in"], inputs["b_in"],
        inputs["W_ih"], inputs["W_hh"], inputs["b_ih"], inputs["b_hh"],
        inputs["W_out"], inputs["b_out"])
    res = run_bass_kernel_spmd(nc, in_maps, core_ids=list(range(NCORES)),
                               trace=trace)
    # per-core out: [TPRED, 2, BL] -> full [B, TPRED, 2]
    full = np.empty((B, TPRED, 2), dtype=np.float32)
    for c in range(NCORES):
        o = res.results[c]["out"]                      # [12, 2, BL]
        full[c * BL:(c + 1) * BL] = np.transpose(o, (2, 0, 1))
    full += inputs["b_out"].astype(np.float32)[None, None, :]
    return full, res


def kernel(**inputs) -> np.ndarray:
    out, _ = _run(inputs, trace=False)
    return out


def _setup_tracing():
    """Register the NTFF profile hook shim (test harness only)."""
    import types

    import trn_agent_boot.trn_boot as tb

    mod = types.ModuleType("antenv.axon_hooks")
    hook = [tb._ntff_profile_via_ctypes("/opt/axon/libaxon_pjrt.so")]
    mod.get_axon_ntff_profile_hook = lambda: hook[0]
    mod.set_axon_ntff_profile_hook = lambda h: hook.__setitem__(0, h)
    sys.modules["antenv.axon_hooks"] = mod
    import antenv
    antenv.axon_hooks = mod

    from concourse import bass_utils
    bass_utils.upload_artifacts = lambda tmpdir: str(tmpdir)


# revision 14
# speedup vs baseline: 1.0060x; 1.0022x over previous
"""Trainium2 Bass kernel for nn_Decoder_31370441129997.

GRU decoder: 12 sequential steps of (Linear+ReLU) -> 3x GRU cell -> Linear(2),
with the input-layer representation fed back from the last GRU layer's hidden.

Strategy: data-parallel over batch (4096 -> 8 cores x 512). All weights
resident in SBUF as bf16 (full PE rate + fast weight loads). Activations kept
feature-major [H, B] so the recurrence needs no transposes. Hidden state is
ping-pong buffered per step parity, so gate outputs write their h tile
directly and no copies are needed. The h-side matmuls (which depend only on
h(t-1)) are software-pipelined two chunks ahead of the x-side matmuls to keep
the PE busy across the gate-latency stalls at layer and step boundaries.
Gate math: ACT sigmoid/tanh with per-partition bias APs (bf16 outputs), DVE
scalar_tensor_tensor for the n-gate preact, GpSimd for the z*h term.
"""
import os
import sys

sys.path.insert(0, "/opt/trn_rl_repo")

from contextlib import ExitStack

import numpy as np

import concourse.bass as bass
import concourse.tile as tile
from concourse import bacc, mybir
from concourse.bass_utils import run_bass_kernel_spmd

TPRED = 12
H = 512
L = 3
B = 4096
NCORES = 8
BL = B // NCORES  # 512 batch rows per core
KT = H // 128     # contraction chunks
MT = H // 128     # feature tiles per gate

F32 = mybir.dt.float32
F32R = mybir.dt.float32r
BF16 = mybir.dt.bfloat16
AF = mybir.ActivationFunctionType
ALU = mybir.AluOpType

# compute dtype for matmul operands: bf16 (~6e-3 error, tolerance is 2e-2)
# or f32r (TF32-like, ~4e-4 error, slower weight loads)
MODE = os.environ.get("KERNEL_DTYPE", "bf16")
CDT = F32R if MODE == "f32r" else BF16
GDT = F32 if MODE == "f32r" else BF16  # gate-value dtype

_CACHE = {}


def _round_f32r(x: np.ndarray) -> np.ndarray:
    """Round fp32 to the PE's float32r grid (RNE, drop 12 low mantissa bits)."""
    u = np.ascontiguousarray(x, dtype=np.float32).view(np.uint32).astype(np.uint64)
    lsb = (u >> np.uint64(12)) & np.uint64(1)
    u = (u + np.uint64(0x7FF) + lsb) & np.uint64(0xFFFFF000)
    return u.astype(np.uint32).view(np.float32).reshape(x.shape)


def _f(t):
    """bitcast f32r tiles to f32 for vector-engine reads."""
    return t[:].bitcast(F32) if CDT is F32R else t[:]


def _build():
    """Build + compile the per-core Bass program (identical on all 8 cores)."""
    nc = bacc.Bacc("TRN2", target_bir_lowering=False, debug=False,
                   dynamic_dma_scratch_size=512)

    rep_d = nc.dram_tensor("rep", [H, BL], CDT, kind="ExternalInput").ap()
    win_d = nc.dram_tensor("win", [128, KT * H], CDT, kind="ExternalInput").ap()
    wx_d = nc.dram_tensor("wx", [L, H, 3 * H], CDT, kind="ExternalInput").ap()
    wh_d = nc.dram_tensor("wh", [L, H, 3 * H], CDT, kind="ExternalInput").ap()
    wout_d = nc.dram_tensor("wout", [H, 2], CDT, kind="ExternalInput").ap()
    bias_d = nc.dram_tensor("bias", [128, 53], F32, kind="ExternalInput").ap()
    out_d = nc.dram_tensor("out", [TPRED, 2, BL], F32, kind="ExternalOutput").ap()

    with tile.TileContext(nc) as tc, ExitStack() as ctx:
        wpool = ctx.enter_context(tc.tile_pool(name="w", bufs=1))
        state = ctx.enter_context(tc.tile_pool(name="state", bufs=1))
        gates = ctx.enter_context(tc.tile_pool(name="gates", bufs=2))
        psum = ctx.enter_context(tc.tile_pool(name="psum", bufs=2, space="PSUM"))

        # ping-pong hidden state: step t writes hb[t%2], reads hb[(t+1)%2].
        # hb[1][2] doubles as the step-0 representation input.
        hb = [[[state.tile([128, BL], CDT, tag=f"h{p}_{l}_{m}",
                           name=f"h{p}_{l}_{m}")
                for m in range(MT)] for l in range(L)] for p in range(2)]

        # Each engine owns one DMA queue moving ~130 GB/s, and descriptor
        # issue costs ~645ns on the issuing engine, so the startup stream is
        # spread need-ordered across all three DMA-capable engines (sync,
        # scalar, gpsimd) to get step 0's rep/W_in/W_ih[0] in flight at once.
        wx = [wpool.tile([128, KT, 3 * H], CDT, tag=f"wx{l}", name=f"wx{l}")
              for l in range(L)]
        wh = [wpool.tile([128, KT, 3 * H], CDT, tag=f"wh{l}", name=f"wh{l}")
              for l in range(L)]
        wout = wpool.tile([128, KT, 2], CDT, tag="wout")
        scratch = state.tile([128, BL], CDT, tag="scratch")

        nc.gpsimd.memset(scratch[:], 0.0)
        for k in (2, 3):
            nc.gpsimd.dma_start(wx[0][:, k, :], wx_d[0, k * 128:(k + 1) * 128, :])
        nc.gpsimd.dma_start(wx[1][:, 3, :], wx_d[1, 3 * 128:4 * 128, :])

        for k in (0, 1):
            nc.scalar.dma_start(wx[0][:, k, :], wx_d[0, k * 128:(k + 1) * 128, :])
        for k in range(KT):
            nc.scalar.dma_start(wx[2][:, k, :], wx_d[2, k * 128:(k + 1) * 128, :])

        # bias first (tiny), then (rep chunk k, win chunk k) pairs so the
        # input-layer matmuls unblock progressively as transfers land
        bias = wpool.tile([128, 53], F32, tag="bias")
        nc.sync.dma_start(bias[:], bias_d[:])
        win = wpool.tile([128, KT, H], CDT, tag="win")
        for k in range(KT):
            nc.sync.dma_start(hb[1][2][k][:], rep_d[k * 128:(k + 1) * 128, :])
            nc.sync.dma_start(win[:, k, :],
                              win_d[:, k * H:(k + 1) * H])
        for k in range(KT - 1):
            nc.sync.dma_start(wx[1][:, k, :], wx_d[1, k * 128:(k + 1) * 128, :])
        nc.sync.dma_start(wout[:], wout_d.rearrange("(kt p) c -> p kt c", p=128))
        for l in range(L):
            for k in range(KT):
                nc.sync.dma_start(wh[l][:, k, :], wh_d[l, k * 128:(k + 1) * 128, :])

        # warm up the PE clock (DVFS ramps over ~3us of continuous work)
        # while the first DMAs are still in flight
        for _ in range(36):
            pw = psum.tile([128, BL], F32, tag="in")
            nc.tensor.matmul(pw[:], lhsT=scratch[:, 0:128], rhs=scratch[:],
                             start=True, stop=True)

        x = [state.tile([128, BL], CDT, tag=f"x{m}", name=f"x{m}")
             for m in range(MT)]

        def bcol(c):
            return bias[:, c:c + 1]

        def outproj(t):
            # b_out is added host-side after the gather
            po = psum.tile([2, BL], F32, tag="z")
            h2 = hb[t % 2][2]
            for k in range(KT):
                nc.tensor.matmul(po[:], lhsT=wout[:, k, :], rhs=h2[k][:],
                                 start=(k == 0), stop=(k == KT - 1))
            o = gates.tile([2, BL], F32, tag="o")
            nc.scalar.copy(o[:], po[:])
            nc.sync.dma_start(out_d[t], o[:])

        def hside(t, l, m):
            """h-side matmul group for (layer l, chunk m): depends only on
            h(t-1), so it is the fill-in work for gate-latency stalls."""
            hp, whl = hb[(t + 1) % 2][l], wh[l]
            lo, hi = m * 128, (m + 1) * 128
            ph = psum.tile([128, BL], F32, tag="hn", name=f"ph_{t}_{l}_{m}")
            for k in range(KT):
                nc.tensor.matmul(ph[:], lhsT=whl[:, k, 2 * H + lo:2 * H + hi],
                                 rhs=hp[k][:], start=(k == 0), stop=(k == KT - 1))
            pr = psum.tile([128, BL], F32, tag="r", name=f"pr_{t}_{l}_{m}")
            for k in range(KT):
                nc.tensor.matmul(pr[:], lhsT=whl[:, k, lo:hi],
                                 rhs=hp[k][:], start=(k == 0), stop=False)
            pz = psum.tile([128, BL], F32, tag="z", name=f"pz_{t}_{l}_{m}")
            for k in range(KT):
                nc.tensor.matmul(pz[:], lhsT=whl[:, k, H + lo:H + hi],
                                 rhs=hp[k][:], start=(k == 0), stop=False)
            return ph, pr, pz

        for t in range(TPRED):
            pend = {}
            if t > 0:
                # two-chunk hoist across the step boundary: pure h(t-1) work
                # that covers the wait for step t-1's last gate chain
                pend[(0, 0)] = hside(t, 0, 0)
                pend[(0, 1)] = hside(t, 0, 1)
                outproj(t - 1)
            # input layer: x = relu(W_in @ h2 + b_in)
            h2in = hb[(t + 1) % 2][2]
            for m in range(MT):
                px = psum.tile([128, BL], F32, tag="in")
                for k in range(KT):
                    nc.tensor.matmul(px[:],
                                     lhsT=win[:, k, m * 128:(m + 1) * 128],
                                     rhs=h2in[k][:],
                                     start=(k == 0), stop=(k == KT - 1))
                nc.scalar.activation(x[m][:], px[:], AF.Relu, bias=bcol(48 + m))
            for l in range(L):
                xin = x if l == 0 else hb[t % 2][l - 1]
                hold = hb[(t + 1) % 2][l]
                wxl = wx[l]
                for m in range(MT):
                    lo = m * 128
                    hi = lo + 128
                    if t > 0:
                        ph, pr, pz = pend.pop((l, m))
                    else:
                        ph = None
                        pr = psum.tile([128, BL], F32, tag="r",
                                       name=f"pr_{t}_{l}_{m}")
                        pz = psum.tile([128, BL], F32, tag="z",
                                       name=f"pz_{t}_{l}_{m}")
                    for k in range(KT):
                        nc.tensor.matmul(pr[:], lhsT=wxl[:, k, lo:hi],
                                         rhs=xin[k][:],
                                         start=(t == 0 and k == 0),
                                         stop=(k == KT - 1))
                    for k in range(KT):
                        nc.tensor.matmul(pz[:], lhsT=wxl[:, k, H + lo:H + hi],
                                         rhs=xin[k][:],
                                         start=(t == 0 and k == 0),
                                         stop=(k == KT - 1))
                    pin = psum.tile([128, BL], F32, tag="in")
                    for k in range(KT):
                        nc.tensor.matmul(pin[:], lhsT=wxl[:, k, 2 * H + lo:2 * H + hi],
                                         rhs=xin[k][:], start=(k == 0),
                                         stop=(k == KT - 1))
                    # keep the PE two h-side chunks ahead of the x-side: the
                    # next layer's first x-side matmul stalls on this layer's
                    # last gate chain, so 24 insts of h(t-1) work go right
                    # before it (the step boundary gets the same treatment
                    # from the next iteration's top-of-loop hoist).
                    if t > 0:
                        if m < 2:
                            pend[(l, m + 2)] = hside(t, l, m + 2)
                        elif m == 3 and l + 1 < L:
                            pend[(l + 1, 0)] = hside(t, l + 1, 0)
                            pend[(l + 1, 1)] = hside(t, l + 1, 1)

                    # gates for this feature chunk.
                    # h' = e1 + q*n with e1 = z*h, q = 1-z hoisted off the
                    # post-tanh critical chain (t=0: h' = n - z*n).
                    r = gates.tile([128, BL], GDT, tag="r")
                    nc.scalar.activation(r[:], pr[:], AF.Sigmoid,
                                         bias=bcol(l * 16 + m))
                    z = gates.tile([128, BL], GDT, tag="z")
                    nc.scalar.activation(z[:], pz[:], AF.Sigmoid,
                                         bias=bcol(l * 16 + 4 + m))
                    if t > 0:
                        q = gates.tile([128, BL], GDT, tag="q")
                        nc.scalar.activation(q[:], z[:], AF.Identity, bias=1.0,
                                             scale=-1.0)
                        e1 = gates.tile([128, BL], GDT, tag="e1")
                        nc.gpsimd.tensor_mul(e1[:], z[:], _f(hold[m]))
                    t1 = gates.tile([128, BL], F32, tag="t1")
                    if t > 0:
                        # t1 = (hn_psum + b_hh_n) * r
                        nc.vector.scalar_tensor_tensor(
                            t1[:], ph[:], bcol(l * 16 + 8 + m), r[:],
                            op0=ALU.add, op1=ALU.mult)
                    else:
                        nc.vector.tensor_scalar(t1[:], r[:], bcol(l * 16 + 8 + m),
                                                None, op0=ALU.mult)
                    t2 = gates.tile([128, BL], F32, tag="t2")
                    nc.vector.tensor_add(t2[:], t1[:], pin[:])
                    n = gates.tile([128, BL], GDT, tag="n")
                    nc.scalar.activation(n[:], t2[:], AF.Tanh,
                                         bias=bcol(l * 16 + 12 + m))
                    e2 = gates.tile([128, BL], GDT, tag="e2")
                    hdst = hb[t % 2][l][m]
                    if t > 0:
                        nc.vector.tensor_mul(e2[:], q[:], n[:])
                        nc.vector.tensor_add(hdst[:], e1[:], e2[:])
                    else:
                        nc.vector.tensor_mul(e2[:], z[:], n[:])
                        nc.vector.tensor_sub(hdst[:], n[:], e2[:])
        outproj(TPRED - 1)

    nc.compile()
    return nc


def _to_dev(x):
    if CDT is F32R:
        return _round_f32r(x)
    import ml_dtypes
    return np.ascontiguousarray(x).astype(ml_dtypes.bfloat16)


def _prep_inputs(representation, W_in, b_in, W_ih, W_hh, b_ih, b_hh, W_out, b_out):
    rep_T = np.ascontiguousarray(representation.reshape(B, H).T)  # [H, B]
    win = _to_dev(np.ascontiguousarray(
        W_in.T.reshape(KT, 128, H).transpose(1, 0, 2).reshape(128, KT * H)))
    wx = _to_dev(np.ascontiguousarray(np.transpose(W_ih, (0, 2, 1))))
    wh = _to_dev(np.ascontiguousarray(np.transpose(W_hh, (0, 2, 1))))
    wout = _to_dev(np.ascontiguousarray(W_out.T))                 # [H, 2]

    bias = np.zeros((128, 53), dtype=np.float32)
    brz = (b_ih[:, :2 * H] + b_hh[:, :2 * H]).astype(np.float32)  # [L, 2H]
    for l in range(L):
        for g in range(2):
            for m in range(MT):
                bias[:, l * 16 + g * 4 + m] = brz[l, g * H + m * 128:
                                                  g * H + (m + 1) * 128]
        for m in range(MT):
            bias[:, l * 16 + 8 + m] = b_hh[l, 2 * H + m * 128:2 * H + (m + 1) * 128]
            bias[:, l * 16 + 12 + m] = b_ih[l, 2 * H + m * 128:2 * H + (m + 1) * 128]
    for m in range(MT):
        bias[:, 48 + m] = b_in[m * 128:(m + 1) * 128]
    bias[0:2, 52] = b_out

    shared = {"win": win, "wx": wx, "wh": wh, "wout": wout, "bias": bias}
    in_maps = []
    for c in range(NCORES):
        m = dict(shared)
        m["rep"] = _to_dev(np.ascontiguousarray(rep_T[:, c * BL:(c + 1) * BL]))
        in_maps.append(m)
    return in_maps


def _run(inputs, trace=False):
    if "nc" not in _CACHE:
        _CACHE["nc"] = _build()
    nc = _CACHE["nc"]
    in_maps = _prep_inputs(
        inputs["representation"], inputs["W_in"], inputs["b_in"],
        inputs["W_ih"], inputs["W_hh"], inputs["b_ih"], inputs["b_hh"],
        inputs["W_out"], inputs["b_out"])
    res = run_bass_kernel_spmd(nc, in_maps, core_ids=list(range(NCORES)),
                               trace=trace)
    # per-core out: [TPRED, 2, BL] -> full [B, TPRED, 2]
    full = np.empty((B, TPRED, 2), dtype=np.float32)
    for c in range(NCORES):
        o = res.results[c]["out"]                      # [12, 2, BL]
        full[c * BL:(c + 1) * BL] = np.transpose(o, (2, 0, 1))
    full += inputs["b_out"].astype(np.float32)[None, None, :]
    return full, res


def kernel(**inputs) -> np.ndarray:
    out, _ = _run(inputs, trace=False)
    return out


def _setup_tracing():
    """Register the NTFF profile hook shim (test harness only)."""
    import types

    import trn_agent_boot.trn_boot as tb

    mod = types.ModuleType("antenv.axon_hooks")
    hook = [tb._ntff_profile_via_ctypes("/opt/axon/libaxon_pjrt.so")]
    mod.get_axon_ntff_profile_hook = lambda: hook[0]
    mod.set_axon_ntff_profile_hook = lambda h: hook.__setitem__(0, h)
    sys.modules["antenv.axon_hooks"] = mod
    import antenv
    antenv.axon_hooks = mod

    from concourse import bass_utils
    bass_utils.upload_artifacts = lambda tmpdir: str(tmpdir)


# revision 18
# speedup vs baseline: 1.0063x; 1.0003x over previous
"""Trainium2 Bass kernel for nn_Decoder_31370441129997.

GRU decoder: 12 sequential steps of (Linear+ReLU) -> 3x GRU cell -> Linear(2),
with the input-layer representation fed back from the last GRU layer's hidden.

Strategy: data-parallel over batch (4096 -> 8 cores x 512). All weights
resident in SBUF as bf16 (full PE rate + fast weight loads). Activations kept
feature-major [H, B] so the recurrence needs no transposes. Hidden state is
ping-pong buffered per step parity, so gate outputs write their h tile
directly and no copies are needed. The h-side matmuls (which depend only on
h(t-1)) are software-pipelined two chunks ahead of the x-side matmuls to keep
the PE busy across the gate-latency stalls at layer and step boundaries.
Gate math: ACT sigmoid/tanh with per-partition bias APs (bf16 outputs), DVE
scalar_tensor_tensor for the n-gate preact, GpSimd for the z*h term.
"""
import os
import sys

sys.path.insert(0, "/opt/trn_rl_repo")

from contextlib import ExitStack

import numpy as np

import concourse.bass as bass
import concourse.tile as tile
from concourse import bacc, mybir
from concourse.bass_utils import run_bass_kernel_spmd

TPRED = 12
H = 512
L = 3
B = 4096
NCORES = 8
BL = B // NCORES  # 512 batch rows per core
KT = H // 128     # contraction chunks
MT = H // 128     # feature tiles per gate

F32 = mybir.dt.float32
F32R = mybir.dt.float32r
BF16 = mybir.dt.bfloat16
AF = mybir.ActivationFunctionType
ALU = mybir.AluOpType

# compute dtype for matmul operands: bf16 (~6e-3 error, tolerance is 2e-2)
# or f32r (TF32-like, ~4e-4 error, slower weight loads)
MODE = os.environ.get("KERNEL_DTYPE", "bf16")
CDT = F32R if MODE == "f32r" else BF16
GDT = F32 if MODE == "f32r" else BF16  # gate-value dtype

_CACHE = {}


def _round_f32r(x: np.ndarray) -> np.ndarray:
    """Round fp32 to the PE's float32r grid (RNE, drop 12 low mantissa bits)."""
    u = np.ascontiguousarray(x, dtype=np.float32).view(np.uint32).astype(np.uint64)
    lsb = (u >> np.uint64(12)) & np.uint64(1)
    u = (u + np.uint64(0x7FF) + lsb) & np.uint64(0xFFFFF000)
    return u.astype(np.uint32).view(np.float32).reshape(x.shape)


def _f(t):
    """bitcast f32r tiles to f32 for vector-engine reads."""
    return t[:].bitcast(F32) if CDT is F32R else t[:]


def _build():
    """Build + compile the per-core Bass program (identical on all 8 cores)."""
    nc = bacc.Bacc("TRN2", target_bir_lowering=False, debug=False,
                   dynamic_dma_scratch_size=512)

    rep_d = nc.dram_tensor("rep", [H, BL], CDT, kind="ExternalInput").ap()
    win_d = nc.dram_tensor("win", [128, KT * H], CDT, kind="ExternalInput").ap()
    wx_d = nc.dram_tensor("wx", [L, H, 3 * H], CDT, kind="ExternalInput").ap()
    wh_d = nc.dram_tensor("wh", [L, H, 3 * H], CDT, kind="ExternalInput").ap()
    wout_d = nc.dram_tensor("wout", [H, 2], CDT, kind="ExternalInput").ap()
    bias_d = nc.dram_tensor("bias", [128, 53], F32, kind="ExternalInput").ap()
    out_d = nc.dram_tensor("out", [TPRED, 2, BL], F32, kind="ExternalOutput").ap()

    with tile.TileContext(nc) as tc, ExitStack() as ctx:
        wpool = ctx.enter_context(tc.tile_pool(name="w", bufs=1))
        state = ctx.enter_context(tc.tile_pool(name="state", bufs=1))
        gates = ctx.enter_context(tc.tile_pool(name="gates", bufs=2))
        psum = ctx.enter_context(tc.tile_pool(name="psum", bufs=2, space="PSUM"))

        # ping-pong hidden state: step t writes hb[t%2], reads hb[(t+1)%2].
        # hb[1][2] doubles as the step-0 representation input.
        hb = [[[state.tile([128, BL], CDT, tag=f"h{p}_{l}_{m}",
                           name=f"h{p}_{l}_{m}")
                for m in range(MT)] for l in range(L)] for p in range(2)]

        # Each engine owns one DMA queue moving ~130 GB/s, and descriptor
        # issue costs ~645ns on the issuing engine, so the startup stream is
        # spread need-ordered across all three DMA-capable engines (sync,
        # scalar, gpsimd) to get step 0's rep/W_in/W_ih[0] in flight at once.
        wx = [wpool.tile([128, KT, 3 * H], CDT, tag=f"wx{l}", name=f"wx{l}")
              for l in range(L)]
        wh = [wpool.tile([128, KT, 3 * H], CDT, tag=f"wh{l}", name=f"wh{l}")
              for l in range(L)]
        wout = wpool.tile([128, KT, 2], CDT, tag="wout")
        scratch = state.tile([128, BL], CDT, tag="scratch")

        nc.gpsimd.memset(scratch[:], 0.0)
        for k in (2, 3):
            nc.gpsimd.dma_start(wx[0][:, k, :], wx_d[0, k * 128:(k + 1) * 128, :])
        nc.gpsimd.dma_start(wx[1][:, 3, :], wx_d[1, 3 * 128:4 * 128, :])

        for k in (0, 1):
            nc.scalar.dma_start(wx[0][:, k, :], wx_d[0, k * 128:(k + 1) * 128, :])
        for k in range(KT):
            nc.scalar.dma_start(wx[2][:, k, :], wx_d[2, k * 128:(k + 1) * 128, :])

        # bias first (tiny), then (rep chunk k, win chunk k) pairs so the
        # input-layer matmuls unblock progressively as transfers land
        bias = wpool.tile([128, 53], F32, tag="bias")
        nc.sync.dma_start(bias[:], bias_d[:])
        win = wpool.tile([128, KT, H], CDT, tag="win")
        for k in range(KT):
            nc.sync.dma_start(hb[1][2][k][:], rep_d[k * 128:(k + 1) * 128, :])
            nc.sync.dma_start(win[:, k, :],
                              win_d[:, k * H:(k + 1) * H])
        for k in range(KT - 1):
            nc.sync.dma_start(wx[1][:, k, :], wx_d[1, k * 128:(k + 1) * 128, :])
        nc.sync.dma_start(wout[:], wout_d.rearrange("(kt p) c -> p kt c", p=128))
        for l in range(L):
            for k in range(KT):
                nc.sync.dma_start(wh[l][:, k, :], wh_d[l, k * 128:(k + 1) * 128, :])

        # warm up the PE clock (DVFS ramps over ~3us of continuous work)
        # while the first DMAs are still in flight
        for _ in range(36):
            pw = psum.tile([128, BL], F32, tag="in")
            nc.tensor.matmul(pw[:], lhsT=scratch[:, 0:128], rhs=scratch[:],
                             start=True, stop=True)

        x = [state.tile([128, BL], CDT, tag=f"x{m}", name=f"x{m}")
             for m in range(MT)]

        def bcol(c):
            return bias[:, c:c + 1]

        def outproj(t):
            # b_out is added host-side after the gather
            po = psum.tile([2, BL], F32, tag="z")
            h2 = hb[t % 2][2]
            for k in range(KT):
                nc.tensor.matmul(po[:], lhsT=wout[:, k, :], rhs=h2[k][:],
                                 start=(k == 0), stop=(k == KT - 1))
            o = gates.tile([2, BL], F32, tag="o")
            nc.scalar.copy(o[:], po[:])
            nc.sync.dma_start(out_d[t], o[:])

        def hside(t, l, m):
            """h-side matmul group for (layer l, chunk m): depends only on
            h(t-1), so it is the fill-in work for gate-latency stalls."""
            hp, whl = hb[(t + 1) % 2][l], wh[l]
            lo, hi = m * 128, (m + 1) * 128
            ph = psum.tile([128, BL], F32, tag="hn", name=f"ph_{t}_{l}_{m}")
            for k in range(KT):
                nc.tensor.matmul(ph[:], lhsT=whl[:, k, 2 * H + lo:2 * H + hi],
                                 rhs=hp[k][:], start=(k == 0), stop=(k == KT - 1))
            pr = psum.tile([128, BL], F32, tag="r", name=f"pr_{t}_{l}_{m}")
            for k in range(KT):
                nc.tensor.matmul(pr[:], lhsT=whl[:, k, lo:hi],
                                 rhs=hp[k][:], start=(k == 0), stop=False)
            pz = psum.tile([128, BL], F32, tag="z", name=f"pz_{t}_{l}_{m}")
            for k in range(KT):
                nc.tensor.matmul(pz[:], lhsT=whl[:, k, H + lo:H + hi],
                                 rhs=hp[k][:], start=(k == 0), stop=False)
            return ph, pr, pz

        for t in range(TPRED):
            pend = {}
            if t > 0:
                # two-chunk hoist across the step boundary: pure h(t-1) work
                # that covers the wait for step t-1's last gate chain
                pend[(0, 0)] = hside(t, 0, 0)
                pend[(0, 1)] = hside(t, 0, 1)
                outproj(t - 1)
            # input layer: x = relu(W_in @ h2 + b_in)
            h2in = hb[(t + 1) % 2][2]
            for m in range(MT):
                px = psum.tile([128, BL], F32, tag="in")
                for k in range(KT):
                    nc.tensor.matmul(px[:],
                                     lhsT=win[:, k, m * 128:(m + 1) * 128],
                                     rhs=h2in[k][:],
                                     start=(k == 0), stop=(k == KT - 1))
                nc.scalar.activation(x[m][:], px[:], AF.Relu, bias=bcol(48 + m))
            for l in range(L):
                xin = x if l == 0 else hb[t % 2][l - 1]
                hold = hb[(t + 1) % 2][l]
                wxl = wx[l]
                for m in range(MT):
                    lo = m * 128
                    hi = lo + 128
                    if t > 0:
                        ph, pr, pz = pend.pop((l, m))
                    else:
                        ph = None
                        pr = psum.tile([128, BL], F32, tag="r",
                                       name=f"pr_{t}_{l}_{m}")
                        pz = psum.tile([128, BL], F32, tag="z",
                                       name=f"pz_{t}_{l}_{m}")
                    for k in range(KT):
                        nc.tensor.matmul(pr[:], lhsT=wxl[:, k, lo:hi],
                                         rhs=xin[k][:],
                                         start=(t == 0 and k == 0),
                                         stop=(k == KT - 1))
                    for k in range(KT):
                        nc.tensor.matmul(pz[:], lhsT=wxl[:, k, H + lo:H + hi],
                                         rhs=xin[k][:],
                                         start=(t == 0 and k == 0),
                                         stop=(k == KT - 1))
                    pin = psum.tile([128, BL], F32, tag="in")
                    for k in range(KT):
                        nc.tensor.matmul(pin[:], lhsT=wxl[:, k, 2 * H + lo:2 * H + hi],
                                         rhs=xin[k][:], start=(k == 0),
                                         stop=(k == KT - 1))
                    # keep the PE two h-side chunks ahead of the x-side: the
                    # next layer's first x-side matmul stalls on this layer's
                    # last gate chain, so 24 insts of h(t-1) work go right
                    # before it (the step boundary gets the same treatment
                    # from the next iteration's top-of-loop hoist).
                    if t > 0:
                        if m < 2:
                            pend[(l, m + 2)] = hside(t, l, m + 2)
                        elif m == 3 and l + 1 < L:
                            pend[(l + 1, 0)] = hside(t, l + 1, 0)
                            pend[(l + 1, 1)] = hside(t, l + 1, 1)

                    # gates for this feature chunk.
                    # h' = e1 + q*n with e1 = z*h, q = 1-z hoisted off the
                    # post-tanh critical chain (t=0: h' = n - z*n).
                    r = gates.tile([128, BL], GDT, tag="r")
                    nc.scalar.activation(r[:], pr[:], AF.Sigmoid,
                                         bias=bcol(l * 16 + m))
                    z = gates.tile([128, BL], GDT, tag="z")
                    nc.scalar.activation(z[:], pz[:], AF.Sigmoid,
                                         bias=bcol(l * 16 + 4 + m))
                    if t > 0:
                        q = gates.tile([128, BL], GDT, tag="q")
                        nc.scalar.activation(q[:], z[:], AF.Identity, bias=1.0,
                                             scale=-1.0)
                        e1 = gates.tile([128, BL], GDT, tag="e1")
                        nc.gpsimd.tensor_mul(e1[:], z[:], _f(hold[m]))
                    t1 = gates.tile([128, BL], F32, tag="t1")
                    if t > 0:
                        # t1 = (hn_psum + b_hh_n) * r
                        nc.vector.scalar_tensor_tensor(
                            t1[:], ph[:], bcol(l * 16 + 8 + m), r[:],
                            op0=ALU.add, op1=ALU.mult)
                    else:
                        nc.vector.tensor_scalar(t1[:], r[:], bcol(l * 16 + 8 + m),
                                                None, op0=ALU.mult)
                    t2 = gates.tile([128, BL], F32, tag="t2")
                    nc.vector.tensor_add(t2[:], t1[:], pin[:])
                    n = gates.tile([128, BL], GDT, tag="n")
                    nc.scalar.activation(n[:], t2[:], AF.Tanh,
                                         bias=bcol(l * 16 + 12 + m))
                    e2 = gates.tile([128, BL], GDT, tag="e2")
                    hdst = hb[t % 2][l][m]
                    if t > 0:
                        nc.vector.tensor_mul(e2[:], q[:], n[:])
                        nc.vector.tensor_add(hdst[:], e1[:], e2[:])
                    else:
                        nc.vector.tensor_mul(e2[:], z[:], n[:])
                        nc.vector.tensor_sub(hdst[:], n[:], e2[:])
        outproj(TPRED - 1)

    nc.compile()
    return nc


def _to_dev(x):
    if CDT is F32R:
        return _round_f32r(x)
    import ml_dtypes
    return np.ascontiguousarray(x).astype(ml_dtypes.bfloat16)


def _prep_inputs(representation, W_in, b_in, W_ih, W_hh, b_ih, b_hh, W_out, b_out):
    rep_T = np.ascontiguousarray(representation.reshape(B, H).T)  # [H, B]
    win = _to_dev(np.ascontiguousarray(
        W_in.T.reshape(KT, 128, H).transpose(1, 0, 2).reshape(128, KT * H)))
    wx = _to_dev(np.ascontiguousarray(np.transpose(W_ih, (0, 2, 1))))
    wh = _to_dev(np.ascontiguousarray(np.transpose(W_hh, (0, 2, 1))))
    wout = _to_dev(np.ascontiguousarray(W_out.T))                 # [H, 2]

    bias = np.zeros((128, 53), dtype=np.float32)
    brz = (b_ih[:, :2 * H] + b_hh[:, :2 * H]).astype(np.float32)  # [L, 2H]
    for l in range(L):
        for g in range(2):
            for m in range(MT):
                bias[:, l * 16 + g * 4 + m] = brz[l, g * H + m * 128:
                                                  g * H + (m + 1) * 128]
        for m in range(MT):
            bias[:, l * 16 + 8 + m] = b_hh[l, 2 * H + m * 128:2 * H + (m + 1) * 128]
            bias[:, l * 16 + 12 + m] = b_ih[l, 2 * H + m * 128:2 * H + (m + 1) * 128]
    for m in range(MT):
        bias[:, 48 + m] = b_in[m * 128:(m + 1) * 128]
    bias[0:2, 52] = b_out

    shared = {"win": win, "wx": wx, "wh": wh, "wout": wout, "bias": bias}
    in_maps = []
    for c in range(NCORES):
        m = dict(shared)
        m["rep"] = _to_dev(np.ascontiguousarray(rep_T[:, c * BL:(c + 1) * BL]))
        in_maps.append(m)
    return in_maps


def _run(inputs, trace=False):
    if "nc" not in _CACHE:
        _CACHE["nc"] = _build()
    nc = _CACHE["nc"]
    in_maps = _prep_inputs(
        inputs["representation"], inputs["W_in"], inputs["b_in"],
        inputs["W_ih"], inputs["W_hh"], inputs["b_ih"], inputs["b_hh"],
        inputs["W_out"], inputs["b_out"])
    res = run_bass_kernel_spmd(nc, in_maps, core_ids=list(range(NCORES)),
                               trace=trace)
    # per-core out: [TPRED, 2, BL] -> full [B, TPRED, 2]
    full = np.empty((B, TPRED, 2), dtype=np.float32)
    for c in range(NCORES):
        o = res.results[c]["out"]                      # [12, 2, BL]
        full[c * BL:(c + 1) * BL] = np.transpose(o, (2, 0, 1))
    full += inputs["b_out"].astype(np.float32)[None, None, :]
    return full, res


def kernel(**inputs) -> np.ndarray:
    out, _ = _run(inputs, trace=False)
    return out


def _setup_tracing():
    """Register the NTFF profile hook shim (test harness only)."""
    import types

    import trn_agent_boot.trn_boot as tb

    mod = types.ModuleType("antenv.axon_hooks")
    hook = [tb._ntff_profile_via_ctypes("/opt/axon/libaxon_pjrt.so")]
    mod.get_axon_ntff_profile_hook = lambda: hook[0]
    mod.set_axon_ntff_profile_hook = lambda h: hook.__setitem__(0, h)
    sys.modules["antenv.axon_hooks"] = mod
    import antenv
    antenv.axon_hooks = mod

    from concourse import bass_utils
    bass_utils.upload_artifacts = lambda tmpdir: str(tmpdir)
